# revision 5
# baseline (speedup 1.0000x reference)
"""Trainium2 Bass kernel for nn_GCN_23029614641773.

The reference GCN operates on B independent 27-node graphs where every node of
graph i starts with the same feature vector x[i], and only node 0 of each graph
feeds the classifier head. Exploiting linearity of the edge aggregation, the
whole network collapses exactly (up to fp rounding order) to a per-sample MLP:

    y = x @ W0                                  # [B, 1024]
    s = lrelu(y + b0) + 2*lrelu(3y + b0) + lrelu(5y + b0)
      # node 1's in-neighbours {0,2,4,6} have in-degrees {1,3,3,5};
      # 2*lrelu(3y+b0) == lrelu(6y+2*b0) exactly (scaling by 2 is exact).
      # With b0 == 0 (spec fill): s == max(12y, 2.4y) exactly.
    t = s @ W1;  h = lrelu(t + b1)              # [B, 512]
    v = h @ W2;  g = lrelu(v + b2)              # [B, 256]
    out = g @ Wc + bc                           # [B, 1]

Sharding: pure data parallelism, batch split across 8 NeuronCores; each core
holds the full weight set.

Layout on device: activations kept transposed (features on partitions, batch
on the free dim) so every layer is matmul(out_T, lhsT=W_chunk, rhs=act_T) with
K accumulated in PSUM. x is transposed once on-chip via PE transposes.
"""

import os
from contextlib import ExitStack

import numpy as np

import concourse.bacc as bacc
import concourse.bass as bass
import concourse.mybir as mybir
import concourse.tile as tile
from concourse.bass_utils import run_bass_kernel_spmd

F32 = mybir.dt.float32
P = 128
N_CORES = 8
B_FULL = 2048
B = B_FULL // N_CORES  # 256 rows per core
D0, D1, D2, D3 = 1024, 1024, 512, 256
K0, M0 = D0 // P, D1 // P  # 8, 8
K1, M1 = D1 // P, D2 // P  # 8, 4
K2, M2 = D2 // P, D3 // P  # 4, 2
KC = D3 // P  # 2

MM_DT = F32  # matmul input dtype (float32 or float32r)


def _leaky(nc, out_ap, in_ap):
    # out = max(in, 0.2*in) == leaky_relu(in, 0.2), exact in fp32.
    # (Requires in_ap in SBUF: both operands feed one instruction.)
    nc.vector.scalar_tensor_tensor(
        out_ap, in_ap, 0.2, in_ap,
        mybir.AluOpType.mult, mybir.AluOpType.max,
    )


def _leaky_psum(nc, tmp_pool, out_ap, ps_ap):
    # leaky_relu straight out of PSUM: only one non-scalar input may read
    # PSUM, so stage 0.2*ps in SBUF then max against PSUM.
    t = tmp_pool.tile([ps_ap.partition_size(), ps_ap.free_size()], F32,
                      tag="lk")
    nc.vector.tensor_scalar_mul(t[:], ps_ap, 0.2)
    nc.vector.tensor_max(out_ap, ps_ap, t[:])


def _build(zero_bias: bool):
    nc = bacc.Bacc(
        "TRN2", target_bir_lowering=False, debug=False,
        enable_asserts=False, num_devices=N_CORES,
    )

    x_d = nc.dram_tensor("x", [B, D0], F32, kind="ExternalInput").ap()
    w0_d = nc.dram_tensor("W0", [D0, D1], F32, kind="ExternalInput").ap()
    b0_d = nc.dram_tensor("b0", [D1], F32, kind="ExternalInput").ap()
    w1_d = nc.dram_tensor("W1", [D1, D2], F32, kind="ExternalInput").ap()
    b1_d = nc.dram_tensor("b1", [D2], F32, kind="ExternalInput").ap()
    w2_d = nc.dram_tensor("W2", [D2, D3], F32, kind="ExternalInput").ap()
    b2_d = nc.dram_tensor("b2", [D3], F32, kind="ExternalInput").ap()
    wc_d = nc.dram_tensor("Wc", [D3, 1], F32, kind="ExternalInput").ap()
    bc_d = nc.dram_tensor("bc", [1], F32, kind="ExternalInput").ap()
    eye_d = nc.dram_tensor("eye", [P, P], F32, kind="ExternalInput").ap()
    out_d = nc.dram_tensor("out", [1, B], F32, kind="ExternalOutput").ap()

    with ExitStack() as ctx:
        tc = ctx.enter_context(tile.TileContext(nc))
        const = ctx.enter_context(tc.tile_pool(name="const", bufs=1))
        xrow_p = ctx.enter_context(tc.tile_pool(name="xrow", bufs=2))
        xt_p = ctx.enter_context(tc.tile_pool(name="xt", bufs=K0))
        w0_p = ctx.enter_context(tc.tile_pool(name="w0", bufs=K0))
        w1_p = ctx.enter_context(tc.tile_pool(name="w1", bufs=K1))
        w2_p = ctx.enter_context(tc.tile_pool(name="w2", bufs=K2))
        wc_p = ctx.enter_context(tc.tile_pool(name="wc", bufs=KC))
        s_p = ctx.enter_context(tc.tile_pool(name="s", bufs=K1))
        h_p = ctx.enter_context(tc.tile_pool(name="h", bufs=K2))
        g_p = ctx.enter_context(tc.tile_pool(name="g", bufs=KC))
        tmp_p = ctx.enter_context(tc.tile_pool(name="tmp", bufs=4))
        out_p = ctx.enter_context(tc.tile_pool(name="outp", bufs=1))
        tp_ps = ctx.enter_context(tc.tile_pool(name="tp", bufs=2, space="PSUM"))
        mm_ps = ctx.enter_context(tc.tile_pool(name="mm", bufs=4, space="PSUM"))
        cls_ps = ctx.enter_context(tc.tile_pool(name="cls", bufs=1, space="PSUM"))

        # ---- constants / small inputs ----
        eye = const.tile([P, P], F32, tag="eye")
        nc.sync.dma_start(eye[:], eye_d)
        b0t = const.tile([P, M0], F32, tag="b0t")
        nc.sync.dma_start(b0t[:], b0_d.rearrange("(c p) -> p c", p=P))
        b1t = const.tile([P, M1], F32, tag="b1t")
        nc.sync.dma_start(b1t[:], b1_d.rearrange("(c p) -> p c", p=P))
        b2t = const.tile([P, M2], F32, tag="b2t")
        nc.sync.dma_start(b2t[:], b2_d.rearrange("(c p) -> p c", p=P))
        bct = const.tile([1, 1], F32, tag="bct")
        nc.sync.dma_start(bct[:], bc_d.rearrange("(a b) -> a b", a=1))
        if not zero_bias:
            b0t2 = const.tile([P, M0], F32, tag="b0t2")
            nc.scalar.mul(b0t2[:], b0t[:], 2.0)

        # ---- load x rows, weights ----
        xr = []
        for r in range(B // P):
            t = xrow_p.tile([P, D0], F32)
            nc.sync.dma_start(t[:], x_d[r * P:(r + 1) * P, :])
            xr.append(t)
        w0 = []
        for k in range(K0):
            t = w0_p.tile([P, D1], MM_DT)
            nc.sync.dma_start(t[:], w0_d[k * P:(k + 1) * P, :])
            w0.append(t)
        w1 = []
        for k in range(K1):
            t = w1_p.tile([P, D2], MM_DT)
            nc.sync.dma_start(t[:], w1_d[k * P:(k + 1) * P, :])
            w1.append(t)
        w2 = []
        for k in range(K2):
            t = w2_p.tile([P, D3], MM_DT)
            nc.sync.dma_start(t[:], w2_d[k * P:(k + 1) * P, :])
            w2.append(t)
        wc = []
        for k in range(KC):
            t = wc_p.tile([P, 1], MM_DT)
            nc.sync.dma_start(t[:], wc_d[k * P:(k + 1) * P, :])
            wc.append(t)

        # ---- transpose x: [256, 1024] -> 8 tiles [128, 256] ----
        xt = []
        for k in range(K0):
            xtk = xt_p.tile([P, B], MM_DT)
            for r in range(B // P):
                pt = tp_ps.tile([P, P], F32)
                nc.tensor.transpose(pt[:], xr[r][:, k * P:(k + 1) * P], eye[:])
                nc.vector.tensor_copy(xtk[:, r * P:(r + 1) * P], pt[:])
            xt.append(xtk)

        # ---- layer 1: yT[m] = sum_k W0[k,m].T @ xT[k]; s = f(y) ----
        s_tiles = []
        for m in range(M0):
            ps = mm_ps.tile([P, B], F32)
            for k in range(K0):
                nc.tensor.matmul(
                    ps[:], lhsT=w0[k][:, m * P:(m + 1) * P], rhs=xt[k][:],
                    start=(k == 0), stop=(k == K0 - 1),
                )
            s = s_p.tile([P, B], MM_DT)
            if zero_bias:
                # s = lrelu(y) + 2*lrelu(3y) + lrelu(5y) = 12*lrelu(y)
                #   = max(12y, 2.4y)   (exact: lrelu is positively homogeneous)
                t12 = tmp_p.tile([P, B], F32, tag="t12")
                nc.vector.tensor_scalar_mul(t12[:], ps[:], 12.0)
                nc.vector.scalar_tensor_tensor(
                    s[:], ps[:], 2.4, t12[:],
                    mybir.AluOpType.mult, mybir.AluOpType.max,
                )
            else:
                b0c = b0t[:, m:m + 1]
                b0c2 = b0t2[:, m:m + 1]
                acc = None
                for scale, bias in ((1.0, b0c), (6.0, b0c2), (5.0, b0c)):
                    v = tmp_p.tile([P, B], F32, tag="v")
                    nc.vector.tensor_scalar(
                        v[:], ps[:], scale, bias,
                        mybir.AluOpType.mult, mybir.AluOpType.add,
                    )
                    l = tmp_p.tile([P, B], F32, tag="l")
                    _leaky(nc, l[:], v[:])
                    if acc is None:
                        nc.vector.tensor_copy(s[:], l[:])
                    else:
                        nc.vector.tensor_add(s[:], s[:], l[:])
                    acc = True
            s_tiles.append(s)

        # ---- layer 2: tT[m] = sum_k W1[k,m].T @ sT[k]; h = lrelu(t + b1) ----
        h_tiles = []
        for m in range(M1):
            ps = mm_ps.tile([P, B], F32)
            for k in range(K1):
                nc.tensor.matmul(
                    ps[:], lhsT=w1[k][:, m * P:(m + 1) * P], rhs=s_tiles[k][:],
                    start=(k == 0), stop=(k == K1 - 1),
                )
            h = h_p.tile([P, B], MM_DT)
            if zero_bias:
                _leaky_psum(nc, tmp_p, h[:], ps[:])
            else:
                v = tmp_p.tile([P, B], F32, tag="v")
                nc.vector.tensor_scalar_add(v[:], ps[:], b1t[:, m:m + 1])
                _leaky(nc, h[:], v[:])
            h_tiles.append(h)

        # ---- layer 3: vT[m] = sum_k W2[k,m].T @ hT[k]; g = lrelu(v + b2) ----
        g_tiles = []
        for m in range(M2):
            ps = mm_ps.tile([P, B], F32)
            for k in range(K2):
                nc.tensor.matmul(
                    ps[:], lhsT=w2[k][:, m * P:(m + 1) * P], rhs=h_tiles[k][:],
                    start=(k == 0), stop=(k == K2 - 1),
                )
            g = g_p.tile([P, B], MM_DT)
            if zero_bias:
                _leaky_psum(nc, tmp_p, g[:], ps[:])
            else:
                v = tmp_p.tile([P, B], F32, tag="v")
                nc.vector.tensor_scalar_add(v[:], ps[:], b2t[:, m:m + 1])
                _leaky(nc, g[:], v[:])
            g_tiles.append(g)

        # ---- classifier: out[1, B] = sum_k Wc[k].T @ gT[k] (+ bc) ----
        po = cls_ps.tile([1, B], F32)
        for k in range(KC):
            nc.tensor.matmul(
                po[:], lhsT=wc[k][:, 0:1], rhs=g_tiles[k][:],
                start=(k == 0), stop=(k == KC - 1),
            )
        ob = out_p.tile([1, B], F32)
        if zero_bias:
            nc.vector.tensor_copy(ob[:], po[:])
        else:
            nc.vector.tensor_scalar_add(ob[:], po[:], bct[:, 0:1])
        nc.sync.dma_start(out_d, ob[:])

    nc.compile()
    return nc


_CACHE = {}


def _get_nc(zero_bias: bool):
    if zero_bias not in _CACHE:
        _CACHE[zero_bias] = _build(zero_bias)
    return _CACHE[zero_bias]


def _run(inputs, trace=False, **kw):
    def f32(a):
        return np.ascontiguousarray(np.asarray(a), dtype=np.float32)

    x = f32(inputs["x"])
    W0, b0 = f32(inputs["W0"]), f32(inputs["b0"])
    W1, b1 = f32(inputs["W1"]), f32(inputs["b1"])
    W2, b2 = f32(inputs["W2"]), f32(inputs["b2"])
    Wc, bc = f32(inputs["Wc"]), f32(inputs["bc"])
    zero_bias = not (b0.any() or b1.any() or b2.any() or bc.any())
    nc = _get_nc(zero_bias)

    eye = np.eye(P, dtype=np.float32)
    in_maps = []
    for i in range(N_CORES):
        in_maps.append({
            "x": x[i * B:(i + 1) * B],
            "W0": W0, "b0": b0, "W1": W1, "b1": b1,
            "W2": W2, "b2": b2, "Wc": Wc, "bc": bc,
            "eye": eye,
        })
    res = run_bass_kernel_spmd(nc, in_maps, list(range(N_CORES)),
                               trace=trace, **kw)
    out = np.empty((B_FULL, 1), dtype=np.float32)
    for i in range(N_CORES):
        out[i * B:(i + 1) * B, 0] = res.results[i]["out"][0]
    return out, res


def kernel(**inputs) -> np.ndarray:
    out, _ = _run(inputs)
    return out


# revision 8
# speedup vs baseline: 1.5510x; 1.5510x over previous
"""Trainium2 Bass kernel for nn_GCN_23029614641773.

The reference GCN operates on B independent 27-node graphs where every node of
graph i starts with the same feature vector x[i], and only node 0 of each graph
feeds the classifier head. Exploiting linearity of the edge aggregation, the
whole network collapses exactly (up to fp rounding order) to a per-sample MLP:

    y = x @ W0                                  # [B, 1024]
    s = lrelu(y + b0) + 2*lrelu(3y + b0) + lrelu(5y + b0)
      # node 1's in-neighbours {0,2,4,6} have in-degrees {1,3,3,5};
      # 2*lrelu(3y+b0) == lrelu(6y+2*b0) exactly (scaling by 2 is exact).
      # With b0 == 0 (spec fill): s == max(12y, 2.4y) exactly.
    t = s @ W1;  h = lrelu(t + b1)              # [B, 512]
    v = h @ W2;  g = lrelu(v + b2)              # [B, 256]
    out = g @ Wc + bc                           # [B, 1]

Sharding: pure data parallelism, batch split across 8 NeuronCores; each core
holds the full weight set.

Layout on device: activations kept transposed (features on partitions, batch
on the free dim) so every layer is matmul(out_T, lhsT=W_chunk, rhs=act_T) with
K accumulated in PSUM. x is transposed once on-chip via PE transposes.
"""

import os
from contextlib import ExitStack

import numpy as np

import concourse.bacc as bacc
import concourse.bass as bass
import concourse.mybir as mybir
import concourse.tile as tile
from concourse.bass_utils import run_bass_kernel_spmd

F32 = mybir.dt.float32
P = 128
N_CORES = 8
B_FULL = 2048
B = B_FULL // N_CORES  # 256 rows per core
D0, D1, D2, D3 = 1024, 1024, 512, 256
K0, M0 = D0 // P, D1 // P  # 8, 8
K1, M1 = D1 // P, D2 // P  # 8, 4
K2, M2 = D2 // P, D3 // P  # 4, 2
KC = D3 // P  # 2

MM_DT = F32  # SBUF tile dtype for matmul operands
USE_F32R = True  # stream matmuls as float32r (4x faster on TRN2 PE)
F32R = mybir.dt.float32r


def _mm(ap):
    return ap.bitcast(F32R) if USE_F32R else ap


def _leaky(nc, out_ap, in_ap):
    # out = max(in, 0.2*in) == leaky_relu(in, 0.2), exact in fp32.
    # (Requires in_ap in SBUF: both operands feed one instruction.)
    nc.vector.scalar_tensor_tensor(
        out_ap, in_ap, 0.2, in_ap,
        mybir.AluOpType.mult, mybir.AluOpType.max,
    )


def _leaky_psum(nc, tmp_pool, out_ap, ps_ap):
    # leaky_relu straight out of PSUM: only one non-scalar input may read
    # PSUM, so stage 0.2*ps in SBUF then max against PSUM.
    t = tmp_pool.tile([ps_ap.partition_size(), ps_ap.free_size()], F32,
                      tag="lk")
    nc.vector.tensor_scalar_mul(t[:], ps_ap, 0.2)
    nc.vector.tensor_max(out_ap, ps_ap, t[:])


def _build(zero_bias: bool):
    nc = bacc.Bacc(
        "TRN2", target_bir_lowering=False, debug=False,
        enable_asserts=False, num_devices=N_CORES,
    )

    x_d = nc.dram_tensor("x", [B, D0], F32, kind="ExternalInput").ap()
    w0_d = nc.dram_tensor("W0", [D0, D1], F32, kind="ExternalInput").ap()
    b0_d = nc.dram_tensor("b0", [D1], F32, kind="ExternalInput").ap()
    w1_d = nc.dram_tensor("W1", [D1, D2], F32, kind="ExternalInput").ap()
    b1_d = nc.dram_tensor("b1", [D2], F32, kind="ExternalInput").ap()
    w2_d = nc.dram_tensor("W2", [D2, D3], F32, kind="ExternalInput").ap()
    b2_d = nc.dram_tensor("b2", [D3], F32, kind="ExternalInput").ap()
    wc_d = nc.dram_tensor("Wc", [D3, 1], F32, kind="ExternalInput").ap()
    bc_d = nc.dram_tensor("bc", [1], F32, kind="ExternalInput").ap()
    eye_d = nc.dram_tensor("eye", [P, P], F32, kind="ExternalInput").ap()
    out_d = nc.dram_tensor("out", [1, B], F32, kind="ExternalOutput").ap()

    with ExitStack() as ctx:
        tc = ctx.enter_context(tile.TileContext(nc))
        const = ctx.enter_context(tc.tile_pool(name="const", bufs=1))
        xrow_p = ctx.enter_context(tc.tile_pool(name="xrow", bufs=2))
        xt_p = ctx.enter_context(tc.tile_pool(name="xt", bufs=K0))
        w0_p = ctx.enter_context(tc.tile_pool(name="w0", bufs=K0))
        w1_p = ctx.enter_context(tc.tile_pool(name="w1", bufs=K1))
        w2_p = ctx.enter_context(tc.tile_pool(name="w2", bufs=K2))
        wc_p = ctx.enter_context(tc.tile_pool(name="wc", bufs=KC))
        s_p = ctx.enter_context(tc.tile_pool(name="s", bufs=K1))
        h_p = ctx.enter_context(tc.tile_pool(name="h", bufs=K2))
        g_p = ctx.enter_context(tc.tile_pool(name="g", bufs=KC))
        tmp_p = ctx.enter_context(tc.tile_pool(name="tmp", bufs=4))
        out_p = ctx.enter_context(tc.tile_pool(name="outp", bufs=1))
        tp_ps = ctx.enter_context(tc.tile_pool(name="tp", bufs=2, space="PSUM"))
        mm_ps = ctx.enter_context(tc.tile_pool(name="mm", bufs=4, space="PSUM"))
        cls_ps = ctx.enter_context(tc.tile_pool(name="cls", bufs=1, space="PSUM"))

        # ---- constants / small inputs ----
        eye = const.tile([P, P], F32, tag="eye")
        nc.sync.dma_start(eye[:], eye_d)
        b0t = const.tile([P, M0], F32, tag="b0t")
        nc.sync.dma_start(b0t[:], b0_d.rearrange("(c p) -> p c", p=P))
        b1t = const.tile([P, M1], F32, tag="b1t")
        nc.sync.dma_start(b1t[:], b1_d.rearrange("(c p) -> p c", p=P))
        b2t = const.tile([P, M2], F32, tag="b2t")
        nc.sync.dma_start(b2t[:], b2_d.rearrange("(c p) -> p c", p=P))
        bct = const.tile([1, 1], F32, tag="bct")
        nc.sync.dma_start(bct[:], bc_d.rearrange("(a b) -> a b", a=1))
        if not zero_bias:
            b0t2 = const.tile([P, M0], F32, tag="b0t2")
            nc.scalar.mul(b0t2[:], b0t[:], 2.0)

        # ---- load x rows, weights ----
        xr = []
        for r in range(B // P):
            t = xrow_p.tile([P, D0], F32)
            nc.sync.dma_start(t[:], x_d[r * P:(r + 1) * P, :])
            xr.append(t)
        w0 = []
        for k in range(K0):
            t = w0_p.tile([P, D1], MM_DT)
            nc.sync.dma_start(_mm(t[:]), _mm(w0_d[k * P:(k + 1) * P, :]))
            w0.append(t)
        w1 = []
        for k in range(K1):
            t = w1_p.tile([P, D2], MM_DT)
            nc.sync.dma_start(_mm(t[:]), _mm(w1_d[k * P:(k + 1) * P, :]))
            w1.append(t)
        w2 = []
        for k in range(K2):
            t = w2_p.tile([P, D3], MM_DT)
            nc.sync.dma_start(_mm(t[:]), _mm(w2_d[k * P:(k + 1) * P, :]))
            w2.append(t)
        wc = []
        for k in range(KC):
            t = wc_p.tile([P, 1], MM_DT)
            nc.sync.dma_start(_mm(t[:]), _mm(wc_d[k * P:(k + 1) * P, :]))
            wc.append(t)

        # ---- transpose x: [256, 1024] -> 8 tiles [128, 256] ----
        xt = []
        for k in range(K0):
            xtk = xt_p.tile([P, B], MM_DT)
            for r in range(B // P):
                pt = tp_ps.tile([P, P], F32)
                nc.tensor.transpose(pt[:], xr[r][:, k * P:(k + 1) * P], eye[:])
                nc.vector.tensor_copy(_mm(xtk[:, r * P:(r + 1) * P]), pt[:])
            xt.append(xtk)

        # ---- layer 1: yT[m] = sum_k W0[k,m].T @ xT[k]; s = f(y) ----
        s_tiles = []
        for m in range(M0):
            ps = mm_ps.tile([P, B], F32)
            for k in range(K0):
                nc.tensor.matmul(
                    ps[:], lhsT=_mm(w0[k][:, m * P:(m + 1) * P]),
                    rhs=_mm(xt[k][:]),
                    start=(k == 0), stop=(k == K0 - 1),
                )
            s = s_p.tile([P, B], MM_DT)
            if zero_bias:
                # s = lrelu(y) + 2*lrelu(3y) + lrelu(5y) = 12*lrelu(y)
                #   = max(12y, 2.4y)   (exact: lrelu is positively homogeneous)
                t12 = tmp_p.tile([P, B], F32, tag="t12")
                nc.vector.tensor_scalar_mul(t12[:], ps[:], 12.0)
                nc.vector.scalar_tensor_tensor(
                    _mm(s[:]), ps[:], 2.4, t12[:],
                    mybir.AluOpType.mult, mybir.AluOpType.max,
                )
            else:
                b0c = b0t[:, m:m + 1]
                b0c2 = b0t2[:, m:m + 1]
                acc = None
                for scale, bias in ((1.0, b0c), (6.0, b0c2), (5.0, b0c)):
                    v = tmp_p.tile([P, B], F32, tag="v")
                    nc.vector.tensor_scalar(
                        v[:], ps[:], scale, bias,
                        mybir.AluOpType.mult, mybir.AluOpType.add,
                    )
                    l = tmp_p.tile([P, B], F32, tag="l")
                    _leaky(nc, l[:], v[:])
                    if acc is None:
                        nc.vector.tensor_copy(_mm(s[:]), l[:])
                    else:
                        nc.vector.tensor_add(_mm(s[:]), _mm(s[:]), l[:])
                    acc = True
            s_tiles.append(s)

        # ---- layer 2: tT[m] = sum_k W1[k,m].T @ sT[k]; h = lrelu(t + b1) ----
        h_tiles = []
        for m in range(M1):
            ps = mm_ps.tile([P, B], F32)
            for k in range(K1):
                nc.tensor.matmul(
                    ps[:], lhsT=_mm(w1[k][:, m * P:(m + 1) * P]),
                    rhs=_mm(s_tiles[k][:]),
                    start=(k == 0), stop=(k == K1 - 1),
                )
            h = h_p.tile([P, B], MM_DT)
            if zero_bias:
                _leaky_psum(nc, tmp_p, _mm(h[:]), ps[:])
            else:
                v = tmp_p.tile([P, B], F32, tag="v")
                nc.vector.tensor_scalar_add(v[:], ps[:], b1t[:, m:m + 1])
                _leaky(nc, _mm(h[:]), v[:])
            h_tiles.append(h)

        # ---- layer 3: vT[m] = sum_k W2[k,m].T @ hT[k]; g = lrelu(v + b2) ----
        g_tiles = []
        for m in range(M2):
            ps = mm_ps.tile([P, B], F32)
            for k in range(K2):
                nc.tensor.matmul(
                    ps[:], lhsT=_mm(w2[k][:, m * P:(m + 1) * P]),
                    rhs=_mm(h_tiles[k][:]),
                    start=(k == 0), stop=(k == K2 - 1),
                )
            g = g_p.tile([P, B], MM_DT)
            if zero_bias:
                _leaky_psum(nc, tmp_p, _mm(g[:]), ps[:])
            else:
                v = tmp_p.tile([P, B], F32, tag="v")
                nc.vector.tensor_scalar_add(v[:], ps[:], b2t[:, m:m + 1])
                _leaky(nc, _mm(g[:]), v[:])
            g_tiles.append(g)

        # ---- classifier: out[1, B] = sum_k Wc[k].T @ gT[k] (+ bc) ----
        po = cls_ps.tile([1, B], F32)
        for k in range(KC):
            nc.tensor.matmul(
                po[:], lhsT=_mm(wc[k][:, 0:1]), rhs=_mm(g_tiles[k][:]),
                start=(k == 0), stop=(k == KC - 1),
            )
        ob = out_p.tile([1, B], F32)
        if zero_bias:
            nc.vector.tensor_copy(ob[:], po[:])
        else:
            nc.vector.tensor_scalar_add(ob[:], po[:], bct[:, 0:1])
        nc.sync.dma_start(out_d, ob[:])

    nc.compile()
    return nc


_CACHE = {}


def _get_nc(zero_bias: bool):
    if zero_bias not in _CACHE:
        _CACHE[zero_bias] = _build(zero_bias)
    return _CACHE[zero_bias]


def _run(inputs, trace=False, **kw):
    def f32(a):
        return np.ascontiguousarray(np.asarray(a), dtype=np.float32)

    x = f32(inputs["x"])
    W0, b0 = f32(inputs["W0"]), f32(inputs["b0"])
    W1, b1 = f32(inputs["W1"]), f32(inputs["b1"])
    W2, b2 = f32(inputs["W2"]), f32(inputs["b2"])
    Wc, bc = f32(inputs["Wc"]), f32(inputs["bc"])
    zero_bias = not (b0.any() or b1.any() or b2.any() or bc.any())
    nc = _get_nc(zero_bias)

    eye = np.eye(P, dtype=np.float32)
    in_maps = []
    for i in range(N_CORES):
        in_maps.append({
            "x": x[i * B:(i + 1) * B],
            "W0": W0, "b0": b0, "W1": W1, "b1": b1,
            "W2": W2, "b2": b2, "Wc": Wc, "bc": bc,
            "eye": eye,
        })
    res = run_bass_kernel_spmd(nc, in_maps, list(range(N_CORES)),
                               trace=trace, **kw)
    out = np.empty((B_FULL, 1), dtype=np.float32)
    for i in range(N_CORES):
        out[i * B:(i + 1) * B, 0] = res.results[i]["out"][0]
    return out, res


def kernel(**inputs) -> np.ndarray:
    out, _ = _run(inputs)
    return out
